# revision 26
# baseline (speedup 1.0000x reference)
"""Trainium2 Bass kernel v3 for multi-head attention.

Distribution: data parallel over batch (8 cores, 1 element each).

Per-core pipeline (bf16 matmuls, fp32 PSUM):
  * xT [128,6,2048] bf16 via cast DMA + DMA-xbar transpose.
  * qkT [128,12,2048] bf16: q pairs jt 0-5, k pairs 6-11, head pair
    packed 64+64 on partitions.  ScalarE Copy eviction.
  * v5 [128,16,12,64] bf16 per head (plain x @ w_v, no scaling).
  * scores: 2 bf16 matmuls -> sc [128,512] psum per (pair, m, head).
  * exp -> at2 [128,2,512] bf16, column-split: ScalarE exact Exp on
    cols [0,AC); DVE Schraudolph on [AC,512) (i16 = s*SA+SB truncated,
    bitcast bf16).
  * attnV FLIPPED: lhsT = at2 128x128 query block (stationary),
    rhs = v5 [128 keys, 64] -> psum avq [128 queries, 64] per
    (head, qtile), accumulated over the 16 key tiles.  Cost is only
    64 moving columns per matmul (vs 512 unflipped) -- 2x less PE
    time for the attnV stage.  A parallel 1-column matmul against a
    ones vector accumulates the softmax denominator per query.
  * normalize: one DVE reciprocal per pair ([128,8]), then DVE
    tensor_scalar with per-partition scalar (recip) evicts psum ->
    outQ [128,16,768] bf16 (natural [N,C] layout).
  * outQ -> outT [128,6,2048] via SBUF->SBUF DMA-xbar transposes of
    [128,128] blocks (idle DMA hardware; no PE/DVE/Scalar cost).
  * proj: outT^T @ wproj over 6 ct; DVE tensor_tensor (+bias)
    eviction; DMA out.
"""

import math
import os

import numpy as np

import concourse.bass as bass
import concourse.mybir as mybir
from concourse import bacc, bass_utils
from concourse.tile import TileContext

F32 = mybir.dt.float32
BF16 = mybir.dt.bfloat16
I16 = mybir.dt.int16
AF = mybir.ActivationFunctionType
ALU = mybir.AluOpType

B, N, C = 8, 2048, 768
H, HD = 12, 64
P = 128
NT = N // P          # 16 m tiles
CT = C // P          # 6
NCHUNK = 4
QW = N // NCHUNK     # 512
QT = QW // P         # 4 query tiles per chunk

SCALE = HD ** -0.5
LOG2E = 1.4426950408889634
# Schraudolph int16/bf16: i16 = trunc(s*SA16 + SB16); bitcast bf16.
SA16 = 128.0 * LOG2E * SCALE
SB16 = 127.0 * 128.0 - 4.5

AC = int(os.environ.get("K_AC", "266"))   # ScalarE exact-exp columns (of 512)
LAG = int(os.environ.get("K_LAG", "3"))   # attnV(m) emitted at iter m+LAG
# exp mode 0: column-split per head (ScalarE cols [0,AC), DVE rest)
# exp mode 1: head-split (ScalarE = head A exact, DVE = head B Schraudolph)
EXP_MODE = int(os.environ.get("K_EXP_MODE", "1"))


def build_nc() -> bass.Bass:
    nc = bacc.Bacc(None)
    x = nc.declare_dram_parameter("x", [N, C], F32, isOutput=False)
    w_qkv = nc.declare_dram_parameter("w_qkv", [C, 3 * C], F32, isOutput=False)
    w_proj = nc.declare_dram_parameter("w_proj", [C, C], F32, isOutput=False)
    b_proj = nc.declare_dram_parameter("b_proj", [C], F32, isOutput=False)
    out = nc.declare_dram_parameter("out", [N, C], F32, isOutput=True)

    with TileContext(nc) as tc:
        with (
            tc.tile_pool(name="const", bufs=1) as cpool,
            tc.tile_pool(name="dram", bufs=1, space="DRAM") as dpool,
            tc.tile_pool(name="at", bufs=int(os.environ.get("K_AT_BUFS", "5"))) as at_pool,
            tc.tile_pool(name="rec", bufs=int(os.environ.get("K_REC", "3"))) as rec_pool,
            tc.tile_pool(name="fin", bufs=int(os.environ.get("K_FIN", "3"))) as fin_pool,
            tc.tile_pool(name="psc", bufs=int(os.environ.get("K_PSC", "4")),
                         space="PSUM") as psum_sc,
            tc.tile_pool(name="pav", bufs=int(os.environ.get("K_PAV", "2")),
                         space="PSUM") as psum_av,
            tc.tile_pool(name="pden", bufs=1, space="PSUM") as psum_den,
            tc.tile_pool(name="paux", bufs=int(os.environ.get("K_PAUX", "1")),
                         space="PSUM") as psum_aux,
        ):
            # ---- persistent SBUF tensors -------------------------------
            w_qkv_sb = cpool.tile([P, CT, 3 * C], BF16, tag="wqkv")
            wproj_sb = cpool.tile([P, CT, C], BF16, tag="wproj")
            b_bc = cpool.tile([P, C], F32, tag="bias")
            xT = cpool.tile([P, CT, N], BF16, tag="xT")
            qkT = cpool.tile([P, 12, N], BF16, tag="qkT")
            v5 = cpool.tile([P, NT, H, HD], BF16, tag="v5")
            outQ = cpool.tile([P, NT, C], BF16, tag="outQ")
            outT = cpool.tile([P, CT, N], BF16, tag="outT")
            ones1 = cpool.tile([P, 1], BF16, tag="ones")
            den_t = psum_den.tile([P, 2, 8], F32, tag="den")

            # ---- phase 0: loads ----------------------------------------
            nc.vector.memset(ones1[:], 1.0)
            # startup criticals first on the SWDGE queue: the k-columns of
            # w_qkv (first scores need them), then the x cast chain; q/v
            # weight columns, w_proj and bias follow.
            wq_re = w_qkv.rearrange("(o p) j -> p o j", p=P)
            x_bf = dpool.tile([N, C], BF16)
            oq_dram = dpool.tile([N, C], BF16)
            oq_re = oq_dram.rearrange("(a p) c -> p a c", p=P)

            def load_w(lo, hi):
                nc.gpsimd.dma_start(
                    out=w_qkv_sb[:, :, lo:hi], in_=wq_re[:, :, lo:hi]
                )

            HN = N // 2
            # 2-column-group casts: 512B contiguous elements avoid the
            # sub-512B DMA penalty (8.7us total vs 17.5us), then half-N
            # transposes (first halves first) + need-ordered weight loads.
            nc.gpsimd.dma_start(out=x_bf[:, 0:256], in_=x[:, 0:256])
            nc.gpsimd.dma_start(out=x_bf[:, 256:512], in_=x[:, 256:512])
            for ct in (0, 1, 2, 3):
                nc.sync.dma_start_transpose(
                    xT[:, ct, 0:HN], x_bf[0:HN, ct * P : (ct + 1) * P])
            load_w(C, C + P)                  # k pair 0
            nc.gpsimd.dma_start(out=x_bf[:, 512:768], in_=x[:, 512:768])
            for ct in (4, 5):
                nc.sync.dma_start_transpose(
                    xT[:, ct, 0:HN], x_bf[0:HN, ct * P : (ct + 1) * P])
            load_w(0, P)                      # q pair 0
            load_w(2 * C, 2 * C + 512)        # v heads 0-7 (pairs 0-3)
            load_w(C + P, C + 2 * P)          # k pair 1
            load_w(P, 2 * P)                  # q pair 1
            for ct in range(CT):
                nc.sync.dma_start_transpose(
                    xT[:, ct, HN:N], x_bf[HN:N, ct * P : (ct + 1) * P])
            load_w(C + 2 * P, C + 3 * P)      # k pair 2
            load_w(2 * P, C)                  # q pairs 2-5
            load_w(2 * C + 512, 3 * C)        # v heads 8-11
            load_w(C + 3 * P, 2 * C)          # k pairs 3-5
            nc.gpsimd.dma_start(
                out=wproj_sb[:], in_=w_proj.rearrange("(o p) j -> p o j", p=P)
            )
            nc.sync.dma_start(
                out=b_bc[:], in_=b_proj[None, :].to_broadcast((P, C))
            )

            # ---- qkv projection emitters -------------------------------
            def emit_qk_group(jt: int, c4: int):
                """qkT[:, jt, c4*QW:...]: q (jt<6) or k (jt>=6) pair."""
                ps = psum_aux.tile([P, 512], F32, tag="aux")
                wcol = jt * P if jt < 6 else C + (jt - 6) * P
                for ct in range(CT):
                    nc.tensor.matmul(
                        ps[:, 0:QW],
                        lhsT=w_qkv_sb[:, ct, wcol : wcol + P],
                        rhs=xT[:, ct, c4 * QW : (c4 + 1) * QW],
                        start=(ct == 0),
                        stop=(ct == CT - 1),
                    )
                nc.scalar.copy(out=qkT[:, jt, c4 * QW : (c4 + 1) * QW],
                               in_=ps[:, 0:QW])

            def emit_v_group(nt: int, half: int):
                """v5[:, nt, h-range, :] = x @ w_v.
                half: 0 = heads 0-7, 1 = heads 8-11, 2 = heads 0-3,
                3 = heads 4-7 (quarter groups for startup)."""
                eo, ew, h0, nh = (
                    (0, 512, 0, 8), (512, 256, 8, 4),
                    (0, 256, 0, 4), (256, 256, 4, 4),
                )[half]
                ps = psum_aux.tile([P, 512], F32, tag="aux")
                for ct in range(CT):
                    nc.tensor.matmul(
                        ps[:, 0:ew],
                        lhsT=xT[:, ct, nt * P : (nt + 1) * P],
                        rhs=w_qkv_sb[:, ct, 2 * C + eo : 2 * C + eo + ew],
                        start=(ct == 0),
                        stop=(ct == CT - 1),
                    )
                nc.scalar.copy(out=v5[:, nt, h0 : h0 + nh, :],
                               in_=ps[:, 0:ew])

            # ---- projection emitter ------------------------------------
            def emit_proj_group(nt: int, half: int):
                eo, ew = ((0, 512), (512, 256))[half]
                ps = psum_aux.tile([P, 512], F32, tag="aux")
                for ct in range(CT):
                    nc.tensor.matmul(
                        ps[:, 0:ew],
                        lhsT=outT[:, ct, nt * P : (nt + 1) * P],
                        rhs=wproj_sb[:, ct, eo : eo + ew],
                        start=(ct == 0),
                        stop=(ct == CT - 1),
                    )
                fs = fin_pool.tile([P, 512], F32, tag="fin")
                nc.vector.tensor_tensor(
                    fs[:, 0:ew], ps[:, 0:ew], b_bc[:, eo : eo + ew], ALU.add,
                )
                nc.sync.dma_start(
                    out=out[nt * P : (nt + 1) * P, eo : eo + ew], in_=fs[:, 0:ew]
                )

            def emit_transposes_qt(qq: int):
                for ct in range(CT):
                    nc.sync.dma_start_transpose(
                        outT[:, ct, qq * P : (qq + 1) * P],
                        outQ[:, qq, ct * P : (ct + 1) * P],
                    )

            def emit_transposes(c: int):
                """outQ rows of chunk c -> outT [128,128] blocks."""
                for qt in range(QT):
                    emit_transposes_qt(c * QT + qt)

            # ---- JIT emission slots ------------------------------------
            emit_qk_group(6, 0)
            emit_qk_group(0, 0)
            emit_v_group(0, 0)
            emit_qk_group(6, 1)
            emit_v_group(1, 0)
            emit_v_group(2, 0)
            emit_v_group(3, 0)

            c0_slots: dict[tuple[int, int], tuple] = {}
            for p in range(1, 5):
                c0_slots[(p, 2)] = ("k", p + 1, 0)
                c0_slots[(p, 5)] = ("k", p + 1, 1)
                c0_slots[(p, 8)] = ("k", p + 1, 2)
                c0_slots[(p, 11)] = ("k", p + 1, 3)
                c0_slots[(p, 14)] = ("q", p + 1, 0)
            c0_slots[(0, 4)] = ("k", 1, 0)
            c0_slots[(0, 7)] = ("k", 0, 2)
            c0_slots[(0, 9)] = ("k", 1, 1)
            c0_slots[(0, 11)] = ("k", 0, 3)
            c0_slots[(0, 13)] = ("k", 1, 2)
            c0_slots[(0, 14)] = ("q", 1, 0)
            c0_slots[(0, 15)] = ("k", 1, 3)
            # v(m, half0) feeds pair 0's own attnV(m) at iter m+LAG, so all
            # of half-0 must be emitted inside pair 0 (v(15,0) just before
            # the pair-0 tail).  half-1 (heads 8-11, consumed from pair 4)
            # spreads over pair 1.
            v_slots: dict[tuple[int, int], tuple] = {}
            vjobs = [(nt, 0) for nt in range(4, NT - 1)]
            vjobs += [(nt, 1) for nt in range(NT)]
            slot_iter = [(0, m) for m in (1, 2, 3, 5, 6, 8, 10, 12, 13, 14,
                                          15)]
            slot_iter += [(1, m) for m in range(NT)]
            for (nt, h), pm in zip(vjobs, slot_iter):
                v_slots[pm] = (nt, h)

            # q prefetch for chunk c+1: pairs 3-5, m in {3, 11}
            qnext_slots = {(3, 3): 0, (3, 11): 1, (4, 3): 2, (4, 11): 3,
                           (5, 3): 4, (5, 11): 5}
            # proj for chunk c-1 during chunk c: pairs 1-4 (transposes for
            # chunk c-1 are only emitted at (c, 0, m=2))
            proj_slots = {}
            pj = 0
            for p in range(1, 5):
                for m in (5, 13):
                    proj_slots[(p, m)] = pj
                    pj += 1

            # ---- main attention loops ----------------------------------
            pending_norm: list[tuple] = []

            def flush_norm():
                while pending_norm:
                    avq_, ds_, c_, p_ = pending_norm.pop(0)
                    rec = rec_pool.tile([P, 8], F32, tag="rec")
                    nc.vector.reciprocal(rec[:], den_t[:, ds_, :])
                    for qt in range(QT):
                        for hh in range(2):
                            h = 2 * p_ + hh
                            j = 4 * hh + qt
                            nc.vector.tensor_scalar(
                                out=outQ[:, c_ * QT + qt,
                                         h * HD : (h + 1) * HD],
                                in0=avq_[:, j, :],
                                scalar1=rec[:, j : j + 1],
                                scalar2=None,
                                op0=ALU.mult,
                            )
                        if p_ == 5:
                            # chunk complete for this qtile: transpose now
                            emit_transposes_qt(c_ * QT + qt)

            for c in range(NCHUNK):
                qsl = slice(c * QW, (c + 1) * QW)
                for p in range(6):
                    t_idx = c * 6 + p
                    # start=True on the FIRST matmul touching each psum bank
                    # marks the whole 2KB zero region pending-zero, so every
                    # other group's first start=False write lands as a fresh
                    # value (hardware zero-region semantics).  Pool-slot reuse
                    # deps order this after the previous pair's normalize.
                    avq = psum_av.tile([P, 8, HD], F32, tag="avq",
                                       name=f"avq{t_idx % 2}")
                    dslot = t_idx % 2
                    at_tiles: dict[int, object] = {}

                    def emit_attnv(m: int):
                        at2 = at_tiles.pop(m)
                        for hh in range(2):
                            for qt in range(QT):
                                j = 4 * hh + qt
                                lhs = at2[:, hh, qt * P : (qt + 1) * P]
                                nc.tensor.matmul(
                                    avq[:, j, :], lhsT=lhs,
                                    rhs=v5[:, m, 2 * p + hh, :],
                                    start=(m == 0 and j == 0),
                                    stop=(m == NT - 1),
                                    skip_group_check=True,
                                )
                                nc.tensor.matmul(
                                    den_t[:, dslot, j : j + 1], lhsT=lhs,
                                    rhs=ones1[:],
                                    start=(m == 0 and j == 0),
                                    stop=(m == NT - 1),
                                    skip_group_check=True,
                                )

                    for m in range(NT):
                        msl = slice(m * P, (m + 1) * P)
                        if c == 0:
                            if (p, m) in v_slots:
                                nt, h = v_slots[(p, m)]
                                emit_v_group(nt, h)
                            if (p, m) in c0_slots:
                                kind, pp, i = c0_slots[(p, m)]
                                emit_qk_group((6 + pp) if kind == "k" else pp, i)
                        if m == 1:
                            flush_norm()
                        if c < NCHUNK - 1 and (p, m) in qnext_slots:
                            emit_qk_group(qnext_slots[(p, m)], c + 1)
                        if c > 0 and (p, m) in proj_slots:
                            j = proj_slots[(p, m)]
                            emit_proj_group(4 * (c - 1) + j // 2, j % 2)
                        # scores for m: one psum bank per head
                        at2 = at_pool.tile([P, 2, QW], BF16, tag="at")
                        at_tiles[m] = at2
                        for hh in range(2):
                            scp = psum_sc.tile([P, QW], F32, tag="sc")
                            nc.tensor.matmul(
                                scp[:],
                                lhsT=qkT[64 * hh : 64 * hh + 64, 6 + p, msl],
                                rhs=qkT[64 * hh : 64 * hh + 64, p, qsl],
                                start=True, stop=True,
                            )
                            if EXP_MODE == 1:
                                # head-split: one op per engine per head
                                if hh == 0:
                                    nc.scalar.activation(
                                        at2[:, 0, :], scp[:],
                                        AF.Exp, scale=SCALE,
                                    )
                                else:
                                    nc.vector.tensor_scalar(
                                        out=at2[:, 1, :].bitcast(I16),
                                        in0=scp[:],
                                        scalar1=SA16, scalar2=SB16,
                                        op0=ALU.mult, op1=ALU.add,
                                    )
                                continue
                            # exp: column-split ScalarE / DVE-schraudolph
                            nc.scalar.activation(
                                at2[:, hh, 0:AC], scp[:, 0:AC],
                                AF.Exp, scale=SCALE,
                            )
                            nc.vector.tensor_scalar(
                                out=at2[:, hh, AC:QW].bitcast(I16),
                                in0=scp[:, AC:QW],
                                scalar1=SA16, scalar2=SB16,
                                op0=ALU.mult, op1=ALU.add,
                            )
                        if m >= LAG:
                            emit_attnv(m - LAG)
                    if c == 0 and p == 0:
                        emit_v_group(NT - 1, 0)
                    for mm in sorted(at_tiles):
                        emit_attnv(mm)
                    pending_norm.append((avq, dslot, c, p))
            flush_norm()
            # tail: proj for the last chunk (transposes already inline)
            for qt in range(QT):
                qq = 4 * (NCHUNK - 1) + qt
                emit_proj_group(qq, 0)
                emit_proj_group(qq, 1)

    nc.compile()
    return nc


_NC_CACHE: list = []


def _get_nc() -> bass.Bass:
    if not _NC_CACHE:
        _NC_CACHE.append(build_nc())
    return _NC_CACHE[0]


def run(inputs: dict, trace: bool = False):
    nc = _get_nc()
    x = np.ascontiguousarray(np.asarray(inputs["x"], dtype=np.float32))
    w_qkv = np.ascontiguousarray(np.asarray(inputs["w_qkv"], dtype=np.float32))
    w_proj = np.ascontiguousarray(np.asarray(inputs["w_proj"], dtype=np.float32))
    b_proj = np.ascontiguousarray(np.asarray(inputs["b_proj"], dtype=np.float32))
    in_maps = [
        {"x": x[i], "w_qkv": w_qkv, "w_proj": w_proj, "b_proj": b_proj}
        for i in range(B)
    ]
    try:
        res = bass_utils.run_bass_kernel_spmd(
            nc, in_maps, core_ids=list(range(B)), trace=trace
        )
    except ModuleNotFoundError:
        res = bass_utils.run_bass_kernel_spmd(
            nc, in_maps, core_ids=list(range(B)), trace=False
        )
    out = np.stack([res.results[i]["out"] for i in range(B)], axis=0)
    return out.astype(np.float32), res.exec_time_ns


def kernel(x, w_qkv, w_proj, b_proj):
    trace = os.environ.get("BASS_KERNEL_TRACE", "0") == "1"
    out, _ = run(
        {"x": x, "w_qkv": w_qkv, "w_proj": w_proj, "b_proj": b_proj}, trace=trace
    )
    return out


# revision 32
# speedup vs baseline: 1.0253x; 1.0253x over previous
"""Trainium2 Bass kernel v3 for multi-head attention.

Distribution: data parallel over batch (8 cores, 1 element each).

Per-core pipeline (bf16 matmuls, fp32 PSUM):
  * xT [128,6,2048] bf16 via cast DMA + DMA-xbar transpose.
  * qkT [128,12,2048] bf16: q pairs jt 0-5, k pairs 6-11, head pair
    packed 64+64 on partitions.  ScalarE Copy eviction.
  * v5 [128,16,12,64] bf16 per head (plain x @ w_v, no scaling).
  * scores: 2 bf16 matmuls -> sc [128,512] psum per (pair, m, head).
  * exp -> at2 [128,2,512] bf16, column-split: ScalarE exact Exp on
    cols [0,AC); DVE Schraudolph on [AC,512) (i16 = s*SA+SB truncated,
    bitcast bf16).
  * attnV FLIPPED: lhsT = at2 128x128 query block (stationary),
    rhs = v5 [128 keys, 64] -> psum avq [128 queries, 64] per
    (head, qtile), accumulated over the 16 key tiles.  Cost is only
    64 moving columns per matmul (vs 512 unflipped) -- 2x less PE
    time for the attnV stage.  A parallel 1-column matmul against a
    ones vector accumulates the softmax denominator per query.
  * normalize: one DVE reciprocal per pair ([128,8]), then DVE
    tensor_scalar with per-partition scalar (recip) evicts psum ->
    outQ [128,16,768] bf16 (natural [N,C] layout).
  * outQ -> outT [128,6,2048] via SBUF->SBUF DMA-xbar transposes of
    [128,128] blocks (idle DMA hardware; no PE/DVE/Scalar cost).
  * proj: outT^T @ wproj over 6 ct; DVE tensor_tensor (+bias)
    eviction; DMA out.
"""

import math
import os

import numpy as np

import concourse.bass as bass
import concourse.mybir as mybir
from concourse import bacc, bass_utils
from concourse.tile import TileContext

F32 = mybir.dt.float32
BF16 = mybir.dt.bfloat16
I16 = mybir.dt.int16
AF = mybir.ActivationFunctionType
ALU = mybir.AluOpType

B, N, C = 8, 2048, 768
H, HD = 12, 64
P = 128
NT = N // P          # 16 m tiles
CT = C // P          # 6
NCHUNK = 4
QW = N // NCHUNK     # 512
QT = QW // P         # 4 query tiles per chunk

SCALE = HD ** -0.5
LOG2E = 1.4426950408889634
# Schraudolph int16/bf16: i16 = trunc(s*SA16 + SB16); bitcast bf16.
SA16 = 128.0 * LOG2E * SCALE
SB16 = 127.0 * 128.0 - 4.5

AC = int(os.environ.get("K_AC", "266"))   # ScalarE exact-exp columns (of 512)
LAG = int(os.environ.get("K_LAG", "3"))   # attnV(m) emitted at iter m+LAG
# exp mode 0: column-split per head (ScalarE cols [0,AC), DVE rest)
# exp mode 1: head-split (ScalarE = head A exact, DVE = head B Schraudolph)
EXP_MODE = int(os.environ.get("K_EXP_MODE", "1"))
NORM_DVE = int(os.environ.get("K_NORM_DVE", "0"))
EVICT_MOD = int(os.environ.get("K_EVICT_MOD", "3"))


def build_nc() -> bass.Bass:
    nc = bacc.Bacc(None)
    x = nc.declare_dram_parameter("x", [N, C], F32, isOutput=False)
    w_qkv = nc.declare_dram_parameter("w_qkv", [C, 3 * C], F32, isOutput=False)
    w_proj = nc.declare_dram_parameter("w_proj", [C, C], F32, isOutput=False)
    b_proj = nc.declare_dram_parameter("b_proj", [C], F32, isOutput=False)
    out = nc.declare_dram_parameter("out", [N, C], F32, isOutput=True)

    with TileContext(nc) as tc:
        with (
            tc.tile_pool(name="const", bufs=1) as cpool,
            tc.tile_pool(name="dram", bufs=1, space="DRAM") as dpool,
            tc.tile_pool(name="at", bufs=int(os.environ.get("K_AT_BUFS", "5"))) as at_pool,
            tc.tile_pool(name="rec", bufs=int(os.environ.get("K_REC", "3"))) as rec_pool,
            tc.tile_pool(name="fin", bufs=int(os.environ.get("K_FIN", "3"))) as fin_pool,
            tc.tile_pool(name="psc", bufs=int(os.environ.get("K_PSC", "4")),
                         space="PSUM") as psum_sc,
            tc.tile_pool(name="pav", bufs=int(os.environ.get("K_PAV", "2")),
                         space="PSUM") as psum_av,
            tc.tile_pool(name="pden", bufs=1, space="PSUM") as psum_den,
            tc.tile_pool(name="paux", bufs=int(os.environ.get("K_PAUX", "1")),
                         space="PSUM") as psum_aux,
        ):
            # ---- persistent SBUF tensors -------------------------------
            w_qkv_sb = cpool.tile([P, CT, 3 * C], BF16, tag="wqkv")
            wproj_sb = cpool.tile([P, CT, C], BF16, tag="wproj")
            b_bc = cpool.tile([P, C], F32, tag="bias")
            xT = cpool.tile([P, CT, N], BF16, tag="xT")
            qkT = cpool.tile([P, 12, N], BF16, tag="qkT")
            v5 = cpool.tile([P, NT, H, HD], BF16, tag="v5")
            outQ = cpool.tile([P, NT, C], BF16, tag="outQ")
            outT = cpool.tile([P, CT, N], BF16, tag="outT")
            ones1 = cpool.tile([P, 1], BF16, tag="ones")
            den_t = psum_den.tile([P, 2, 8], F32, tag="den")

            # ---- phase 0: loads ----------------------------------------
            nc.vector.memset(ones1[:], 1.0)
            # startup criticals first on the SWDGE queue: the k-columns of
            # w_qkv (first scores need them), then the x cast chain; q/v
            # weight columns, w_proj and bias follow.
            wq_re = w_qkv.rearrange("(o p) j -> p o j", p=P)
            x_bf = dpool.tile([N, C], BF16)
            oq_dram = dpool.tile([N, C], BF16)
            oq_re = oq_dram.rearrange("(a p) c -> p a c", p=P)

            def load_w(lo, hi):
                nc.gpsimd.dma_start(
                    out=w_qkv_sb[:, :, lo:hi], in_=wq_re[:, :, lo:hi]
                )

            HN = N // 2
            # 2-column-group casts: 512B contiguous elements avoid the
            # sub-512B DMA penalty (8.7us total vs 17.5us), then half-N
            # transposes (first halves first) + need-ordered weight loads.
            nc.gpsimd.dma_start(out=x_bf[:, 0:256], in_=x[:, 0:256])
            nc.gpsimd.dma_start(out=x_bf[:, 256:512], in_=x[:, 256:512])
            for ct in (0, 1, 2, 3):
                nc.sync.dma_start_transpose(
                    xT[:, ct, 0:HN], x_bf[0:HN, ct * P : (ct + 1) * P])
            load_w(C, C + P)                  # k pair 0
            nc.gpsimd.dma_start(out=x_bf[:, 512:768], in_=x[:, 512:768])
            for ct in (4, 5):
                nc.sync.dma_start_transpose(
                    xT[:, ct, 0:HN], x_bf[0:HN, ct * P : (ct + 1) * P])
            load_w(0, P)                      # q pair 0
            load_w(2 * C, 2 * C + 512)        # v heads 0-7 (pairs 0-3)
            load_w(C + P, C + 2 * P)          # k pair 1
            load_w(P, 2 * P)                  # q pair 1
            for ct in range(CT):
                nc.sync.dma_start_transpose(
                    xT[:, ct, HN:N], x_bf[HN:N, ct * P : (ct + 1) * P])
            load_w(C + 2 * P, C + 3 * P)      # k pair 2
            load_w(2 * P, C)                  # q pairs 2-5
            load_w(2 * C + 512, 3 * C)        # v heads 8-11
            load_w(C + 3 * P, 2 * C)          # k pairs 3-5
            nc.gpsimd.dma_start(
                out=wproj_sb[:], in_=w_proj.rearrange("(o p) j -> p o j", p=P)
            )
            nc.sync.dma_start(
                out=b_bc[:], in_=b_proj[None, :].to_broadcast((P, C))
            )

            # ---- qkv projection emitters -------------------------------
            evict_flip = [0]

            def evict(out_ap, in_ap):
                evict_flip[0] = (evict_flip[0] + 1) % EVICT_MOD
                if evict_flip[0]:
                    nc.scalar.copy(out=out_ap, in_=in_ap)
                else:
                    nc.vector.tensor_copy(out_ap, in_ap)

            def emit_qk_group(jt: int, c4: int):
                """qkT[:, jt, c4*QW:...]: q (jt<6) or k (jt>=6) pair."""
                ps = psum_aux.tile([P, 512], F32, tag="aux")
                wcol = jt * P if jt < 6 else C + (jt - 6) * P
                for ct in range(CT):
                    nc.tensor.matmul(
                        ps[:, 0:QW],
                        lhsT=w_qkv_sb[:, ct, wcol : wcol + P],
                        rhs=xT[:, ct, c4 * QW : (c4 + 1) * QW],
                        start=(ct == 0),
                        stop=(ct == CT - 1),
                    )
                evict(qkT[:, jt, c4 * QW : (c4 + 1) * QW], ps[:, 0:QW])

            def emit_v_group(nt: int, half: int):
                """v5[:, nt, h-range, :] = x @ w_v.
                half: 0 = heads 0-7, 1 = heads 8-11, 2 = heads 0-3,
                3 = heads 4-7 (quarter groups for startup)."""
                eo, ew, h0, nh = (
                    (0, 512, 0, 8), (512, 256, 8, 4),
                    (0, 256, 0, 4), (256, 256, 4, 4),
                )[half]
                ps = psum_aux.tile([P, 512], F32, tag="aux")
                for ct in range(CT):
                    nc.tensor.matmul(
                        ps[:, 0:ew],
                        lhsT=xT[:, ct, nt * P : (nt + 1) * P],
                        rhs=w_qkv_sb[:, ct, 2 * C + eo : 2 * C + eo + ew],
                        start=(ct == 0),
                        stop=(ct == CT - 1),
                    )
                evict(v5[:, nt, h0 : h0 + nh, :], ps[:, 0:ew])

            # ---- projection emitter ------------------------------------
            def emit_proj_group(nt: int, half: int):
                eo, ew = ((0, 512), (512, 256))[half]
                ps = psum_aux.tile([P, 512], F32, tag="aux")
                for ct in range(CT):
                    nc.tensor.matmul(
                        ps[:, 0:ew],
                        lhsT=outT[:, ct, nt * P : (nt + 1) * P],
                        rhs=wproj_sb[:, ct, eo : eo + ew],
                        start=(ct == 0),
                        stop=(ct == CT - 1),
                    )
                fs = fin_pool.tile([P, 512], F32, tag="fin")
                nc.vector.tensor_tensor(
                    fs[:, 0:ew], ps[:, 0:ew], b_bc[:, eo : eo + ew], ALU.add,
                )
                nc.sync.dma_start(
                    out=out[nt * P : (nt + 1) * P, eo : eo + ew], in_=fs[:, 0:ew]
                )

            def emit_transposes_qt(qq: int):
                for ct in range(CT):
                    nc.sync.dma_start_transpose(
                        outT[:, ct, qq * P : (qq + 1) * P],
                        outQ[:, qq, ct * P : (ct + 1) * P],
                    )

            def emit_transposes(c: int):
                """outQ rows of chunk c -> outT [128,128] blocks."""
                for qt in range(QT):
                    emit_transposes_qt(c * QT + qt)

            # ---- JIT emission slots ------------------------------------
            emit_qk_group(6, 0)
            emit_qk_group(0, 0)
            emit_v_group(0, 0)
            emit_qk_group(6, 1)
            emit_v_group(1, 0)
            emit_v_group(2, 0)
            emit_v_group(3, 0)

            c0_slots: dict[tuple[int, int], tuple] = {}
            for p in range(1, 5):
                c0_slots[(p, 2)] = ("k", p + 1, 0)
                c0_slots[(p, 5)] = ("k", p + 1, 1)
                c0_slots[(p, 8)] = ("k", p + 1, 2)
                c0_slots[(p, 11)] = ("k", p + 1, 3)
                c0_slots[(p, 14)] = ("q", p + 1, 0)
            c0_slots[(0, 4)] = ("k", 1, 0)
            c0_slots[(0, 7)] = ("k", 0, 2)
            c0_slots[(0, 9)] = ("k", 1, 1)
            c0_slots[(0, 11)] = ("k", 0, 3)
            c0_slots[(0, 13)] = ("k", 1, 2)
            c0_slots[(0, 14)] = ("q", 1, 0)
            c0_slots[(0, 15)] = ("k", 1, 3)
            # v(m, half0) feeds pair 0's own attnV(m) at iter m+LAG, so all
            # of half-0 must be emitted inside pair 0 (v(15,0) just before
            # the pair-0 tail).  half-1 (heads 8-11, consumed from pair 4)
            # spreads over pair 1.
            v_slots: dict[tuple[int, int], tuple] = {}
            vjobs = [(nt, 0) for nt in range(4, NT - 1)]
            vjobs += [(nt, 1) for nt in range(NT)]
            slot_iter = [(0, m) for m in (1, 2, 3, 5, 6, 8, 10, 12, 13, 14,
                                          15)]
            slot_iter += [(1, m) for m in range(NT)]
            for (nt, h), pm in zip(vjobs, slot_iter):
                v_slots[pm] = (nt, h)

            # q prefetch for chunk c+1: pairs 3-5, m in {3, 11}
            qnext_slots = {(3, 3): 0, (3, 11): 1, (4, 3): 2, (4, 11): 3,
                           (5, 3): 4, (5, 11): 5}
            # proj for chunk c-1 during chunk c: pairs 1-4 (transposes for
            # chunk c-1 are only emitted at (c, 0, m=2))
            proj_slots = {}
            pj = 0
            for p in range(1, 5):
                for m in (5, 13):
                    proj_slots[(p, m)] = pj
                    pj += 1

            # ---- main attention loops ----------------------------------
            pending_norm: list[tuple] = []

            def flush_norm(qt_only=None):
                for item in list(pending_norm):
                    avq_, ds_, c_, p_, rec_box = item
                    if rec_box[0] is None:
                        rec = rec_pool.tile([P, 8], F32, tag="rec")
                        nc.vector.reciprocal(rec[:], den_t[:, ds_, :])
                        rec_box[0] = rec
                    rec = rec_box[0]
                    qts = range(QT) if qt_only is None else [qt_only]
                    for qt in qts:
                        for hh in range(2):
                            h = 2 * p_ + hh
                            j = 4 * hh + qt
                            oq = outQ[:, c_ * QT + qt, h * HD : (h + 1) * HD]
                            if NORM_DVE and hh == 0:
                                nc.vector.tensor_scalar(
                                    out=oq, in0=avq_[:, j, :],
                                    scalar1=rec[:, j : j + 1],
                                    scalar2=None, op0=ALU.mult,
                                )
                            else:
                                # ScalarE Copy with per-partition scale
                                nc.scalar.mul(oq, avq_[:, j, :],
                                              rec[:, j : j + 1])
                        if p_ == 5:
                            # chunk complete for this qtile: transpose now
                            emit_transposes_qt(c_ * QT + qt)
                    if qt_only is None or qt_only == QT - 1:
                        pending_norm.remove(item)

            for c in range(NCHUNK):
                qsl = slice(c * QW, (c + 1) * QW)
                for p in range(6):
                    t_idx = c * 6 + p
                    # start=True on the FIRST matmul touching each psum bank
                    # marks the whole 2KB zero region pending-zero, so every
                    # other group's first start=False write lands as a fresh
                    # value (hardware zero-region semantics).  Pool-slot reuse
                    # deps order this after the previous pair's normalize.
                    avq = psum_av.tile([P, 8, HD], F32, tag="avq",
                                       name=f"avq{t_idx % 2}")
                    dslot = t_idx % 2
                    at_tiles: dict[int, object] = {}

                    def emit_attnv(m: int):
                        at2 = at_tiles.pop(m)
                        for hh in range(2):
                            for qt in range(QT):
                                j = 4 * hh + qt
                                lhs = at2[:, hh, qt * P : (qt + 1) * P]
                                nc.tensor.matmul(
                                    avq[:, j, :], lhsT=lhs,
                                    rhs=v5[:, m, 2 * p + hh, :],
                                    start=(m == 0 and j == 0),
                                    stop=(m == NT - 1),
                                    skip_group_check=True,
                                )
                                nc.tensor.matmul(
                                    den_t[:, dslot, j : j + 1], lhsT=lhs,
                                    rhs=ones1[:],
                                    start=(m == 0 and j == 0),
                                    stop=(m == NT - 1),
                                    skip_group_check=True,
                                )

                    for m in range(NT):
                        msl = slice(m * P, (m + 1) * P)
                        if c == 0:
                            if (p, m) in v_slots:
                                nt, h = v_slots[(p, m)]
                                emit_v_group(nt, h)
                            if (p, m) in c0_slots:
                                kind, pp, i = c0_slots[(p, m)]
                                emit_qk_group((6 + pp) if kind == "k" else pp, i)
                        if m == 1:
                            flush_norm()
                        if c < NCHUNK - 1 and (p, m) in qnext_slots:
                            emit_qk_group(qnext_slots[(p, m)], c + 1)
                        if c > 0 and (p, m) in proj_slots:
                            j = proj_slots[(p, m)]
                            emit_proj_group(4 * (c - 1) + j // 2, j % 2)
                        # scores for m: one psum bank per head
                        at2 = at_pool.tile([P, 2, QW], BF16, tag="at")
                        at_tiles[m] = at2
                        for hh in range(2):
                            scp = psum_sc.tile([P, QW], F32, tag="sc")
                            nc.tensor.matmul(
                                scp[:],
                                lhsT=qkT[64 * hh : 64 * hh + 64, 6 + p, msl],
                                rhs=qkT[64 * hh : 64 * hh + 64, p, qsl],
                                start=True, stop=True,
                            )
                            if EXP_MODE == 1:
                                # head-split: one op per engine per head
                                if hh == 0:
                                    nc.scalar.activation(
                                        at2[:, 0, :], scp[:],
                                        AF.Exp, scale=SCALE,
                                    )
                                else:
                                    nc.vector.tensor_scalar(
                                        out=at2[:, 1, :].bitcast(I16),
                                        in0=scp[:],
                                        scalar1=SA16, scalar2=SB16,
                                        op0=ALU.mult, op1=ALU.add,
                                    )
                                continue
                            # exp: column-split ScalarE / DVE-schraudolph
                            nc.scalar.activation(
                                at2[:, hh, 0:AC], scp[:, 0:AC],
                                AF.Exp, scale=SCALE,
                            )
                            nc.vector.tensor_scalar(
                                out=at2[:, hh, AC:QW].bitcast(I16),
                                in0=scp[:, AC:QW],
                                scalar1=SA16, scalar2=SB16,
                                op0=ALU.mult, op1=ALU.add,
                            )
                        if m >= LAG:
                            emit_attnv(m - LAG)
                    if c == 0 and p == 0:
                        emit_v_group(NT - 1, 0)
                    for mm in sorted(at_tiles):
                        emit_attnv(mm)
                    pending_norm.append((avq, dslot, c, p, [None]))
            flush_norm()
            # tail: proj for the last chunk (transposes already inline)
            for qt in range(QT):
                qq = 4 * (NCHUNK - 1) + qt
                emit_proj_group(qq, 0)
                emit_proj_group(qq, 1)

    nc.compile()
    return nc


_NC_CACHE: list = []


def _get_nc() -> bass.Bass:
    if not _NC_CACHE:
        _NC_CACHE.append(build_nc())
    return _NC_CACHE[0]


def run(inputs: dict, trace: bool = False):
    nc = _get_nc()
    x = np.ascontiguousarray(np.asarray(inputs["x"], dtype=np.float32))
    w_qkv = np.ascontiguousarray(np.asarray(inputs["w_qkv"], dtype=np.float32))
    w_proj = np.ascontiguousarray(np.asarray(inputs["w_proj"], dtype=np.float32))
    b_proj = np.ascontiguousarray(np.asarray(inputs["b_proj"], dtype=np.float32))
    in_maps = [
        {"x": x[i], "w_qkv": w_qkv, "w_proj": w_proj, "b_proj": b_proj}
        for i in range(B)
    ]
    try:
        res = bass_utils.run_bass_kernel_spmd(
            nc, in_maps, core_ids=list(range(B)), trace=trace
        )
    except ModuleNotFoundError:
        res = bass_utils.run_bass_kernel_spmd(
            nc, in_maps, core_ids=list(range(B)), trace=False
        )
    out = np.stack([res.results[i]["out"] for i in range(B)], axis=0)
    return out.astype(np.float32), res.exec_time_ns


def kernel(x, w_qkv, w_proj, b_proj):
    trace = os.environ.get("BASS_KERNEL_TRACE", "0") == "1"
    out, _ = run(
        {"x": x, "w_qkv": w_qkv, "w_proj": w_proj, "b_proj": b_proj}, trace=trace
    )
    return out
